# revision 24
# baseline (speedup 1.0000x reference)
"""Multi-head self-attention (causal) on 8 TRN2 NeuronCores — v2.

Problem (hardcoded): B=2, S=2048, D=1024, H=16 heads, HD=64.
  q,k,v = x@W* + b*; scores = qk^T/sqrt(HD) causal-masked; softmax;
  out = (softmax @ v) @ Wo + bo.

Sharding: 8 cores = 2 batches x 4 head-groups (4 heads each).
Core c handles batch c//4, heads (c%4)*4..(c%4)*4+4 (Megatron-style TP:
Wq/Wk/Wv column-sliced, Wo row-sliced; host sums the 4 partial outputs
per batch and adds bo + bv@Wo -- the bv fold is exact because softmax
rows sum to 1, so attn(v+bv) = attn(v) + bv).

v2 is scheduled around the TRN2 PE p-state ramp: the tensor engine only
reaches 2.4 GHz after ~3us of *gapless* execution and drops to 1.2 GHz
after any idle (the v1 trace shows identical matmuls at 216ns in the
dense projection phase vs 427-585ns in the gappy attention phase).  So
the whole kernel is emitted as ONE dense PE stream:

  - scores for j-tile jt+1 are issued before attn@v of j-tile jt, so
    the PE never sits behind the Scalar-engine exp;
  - v/q/k/out-projection matmul chains are interleaved into the
    attention j-tile loop as elastic "fill" work that plugs the
    remaining exp-lag (ACT needs ~1.01us per j-tile vs ~0.85us of
    attention PE work at full clock);
  - x is host-packed so it loads in four 512-column blocks (8KB
    contiguous per partition row) and the first v/q/k chains start
    after ~1.5MB of DMA instead of ~6MB.

Other changes vs v1: causal masking is a post-exp affine_select on the
(otherwise idle) GpSimd engine instead of -1e9 adds on Vector (exp of
an unmasked score is <=e^6, far from overflow, and zeroing the weight
kills the contribution *and* keeps the ones-column denominator exact);
the softmax denominator reciprocal uses reciprocal_approx_fast (~5x
faster than v1's 3.3us InstReciprocal, 18-bit accurate); PSUM attn
accumulators are evacuated with one quick copy so the banks recycle
fast, and the normalize multiply runs on GpSimd; the output is written
as fp16 (halves the store traffic; partials are ~O(3) so fp16 adds
<0.1% error).
"""

import numpy as np
import ml_dtypes

import concourse.bass as bass
import concourse.mybir as mybir
import concourse.tile as tile
from concourse.alu_op_type import AluOpType

P = 128
S = 2048          # per-core sequence (one batch slice)
D = 1024
CL = 256          # local channels = 4 heads * 64
NH = 4            # local heads
HD = 64
DT = D // P       # 8 contraction chunks
CT = CL // P      # 2 local-channel tiles
ST = S // P       # 16 seq tiles
QG = 4            # 512-wide query groups
XB = 4            # xt DMA column blocks of 512
SCALE = 1.0 / np.sqrt(HD)

F32 = mybir.dt.float32
F16 = mybir.dt.float16
BF16 = mybir.dt.bfloat16
CDT = BF16        # compute dtype for matmul operands


def _legalize_waits(nc: bass.Bass) -> None:
    """Hoist excess sync waits into standalone EventSemaphore instructions.

    The TRN2 ISA holds ONE sync-wait per instruction (two on
    EventSemaphore); Tile's sem-assignment can attach more, which walrus
    rejects with "Too many sync wait commands".  Executing the extra
    waits as same-engine EventSemaphores immediately before the
    instruction is semantically identical.
    """
    esn = 0
    for fn in nc.m.functions:
        for blk in fn.blocks:
            new = []
            for inst in blk.instructions:
                si = inst.sync_info
                cap = 2 if isinstance(inst, mybir.InstEventSemaphore) else 1
                if si is not None and si.on_wait and len(si.on_wait) > cap:
                    waits = list(si.on_wait)
                    extra, keep = waits[:-cap], waits[-cap:]
                    while extra:
                        chunk, extra = extra[:2], extra[2:]
                        esn += 1
                        new.append(mybir.InstEventSemaphore(
                            name=f"eswait{esn}_{inst.name}",
                            engine=inst.engine, ins=[], outs=[],
                            sync_info=mybir.SyncInfo(on_wait=chunk, on_update=[]),
                        ))
                    inst.sync_info = mybir.SyncInfo(
                        on_wait=keep, on_update=list(si.on_update)
                    )
                new.append(inst)
            blk.instructions[:] = new


def build_nc() -> bass.Bass:
    nc = bass.Bass()
    xt = nc.declare_dram_parameter("xt", [P, XB * DT * 512], CDT, isOutput=False)
    wq = nc.declare_dram_parameter("wq", [P, DT * CL], CDT, isOutput=False)
    wk = nc.declare_dram_parameter("wk", [P, DT * CL], CDT, isOutput=False)
    wv = nc.declare_dram_parameter("wv", [P, DT * CL], CDT, isOutput=False)
    wo = nc.declare_dram_parameter("wo", [P, CT * D], CDT, isOutput=False)
    bqk = nc.declare_dram_parameter("bqk", [P, 2 * CT], F32, isOutput=False)
    out = nc.declare_dram_parameter("out", [S, D], F16, isOutput=True)

    with tile.TileContext(nc) as tc:
        with tc.tile_pool(name="const", bufs=1) as const, \
             tc.tile_pool(name="sc_ps", bufs=2, space="PSUM") as sc_pool, \
             tc.tile_pool(name="at_ps", bufs=2, space="PSUM") as at_pool, \
             tc.tile_pool(name="fl_ps", bufs=2, space="PSUM") as fl_pool, \
             tc.tile_pool(name="wtp", bufs=4) as wt_pool, \
             tc.tile_pool(name="smp", bufs=4) as sm_pool, \
             tc.tile_pool(name="osp", bufs=3) as os_pool:

            # persistent SBUF tensors
            # xt is BLOCK-major ([P, 4 col-blocks, 8 k-chunks, 512]) so each
            # block's DMA writes one contiguous 8KB run per partition: the
            # SP sequencer generates 128 descriptors (~0.6us) instead of
            # 1024 (~4.4us) per block, and compute starts ~7us earlier.
            xt_sb = const.tile([P, XB, DT, 512], CDT)
            wq_sb = const.tile([P, DT, CL], CDT)
            wk_sb = const.tile([P, DT, CL], CDT)
            wv_sb = const.tile([P, DT, CL], CDT)
            wo_sb = const.tile([P, CT, D], CDT)
            b_sb = const.tile([P, 2, CT], F32)
            qT_sb = const.tile([P, CT, S], CDT)
            kT_sb = const.tile([P, CT, S], CDT)
            # cols [HD, 2*HD) are all-ones: the attn matmul then emits the
            # softmax denominator replicated on PSUM partitions 64..127.
            v_sb = const.tile([P, ST, NH, 2 * HD], CDT)
            # attnT (normalized), SPLIT per ct: a shared tile made the
            # epilogue's ct0 matmuls inherit a false wait on the final
            # group's ct1 normalize (conservative cross-ct dependency).
            aT0_sb = const.tile([P, S], CDT)
            aT1_sb = const.tile([P, S], CDT)
            aT_ct = (aT0_sb, aT1_sb)
            escr = const.tile([P, 2], F32)                # exp-table preload

            # ---- input DMAs, priority order ----
            # gpsimd = single SWDGE queue for the tiny bias gather.
            b_ld = const.tile([P, 2, CT], F32)
            nc.gpsimd.dma_start(
                out=b_ld[:], in_=bqk.rearrange("p (w c) -> p w c", w=2)
            )
            # first-needed loads first: the prologue v/qk chains need only
            # wv+wq+wk and xt block 0.
            xt_dr = xt.rearrange("p (k t j) -> p k t j", k=XB, t=DT)
            nc.sync.dma_start(
                out=wv_sb[:], in_=wv.rearrange("p (t c) -> p t c", t=DT)
            )
            nc.sync.dma_start(out=xt_sb[:, 0], in_=xt_dr[:, 0])
            for w_sb, w_dr in ((wq_sb, wq), (wk_sb, wk)):
                nc.sync.dma_start(
                    out=w_sb[:], in_=w_dr.rearrange("p (t c) -> p t c", t=DT)
                )
            for k in range(1, XB):
                nc.sync.dma_start(out=xt_sb[:, k], in_=xt_dr[:, k])
            nc.sync.dma_start(
                out=wo_sb[:], in_=wo.rearrange("p (t c) -> p t c", t=CT)
            )

            # TensorScalarPtr holds only ONE sync wait, so absorb the DMA
            # wait into a DVE copy: consumers then only wait on DVE.
            nc.vector.tensor_copy(b_sb[:], b_ld[:])
            nc.vector.memset(v_sb[:, :, :, HD:], 1.0)
            # preload the Exp table while DMAs stream (one-time 1.3us).
            nc.scalar.activation(
                out=escr, in_=b_sb[:, 0, :],
                func=mybir.ActivationFunctionType.Exp,
            )

            # ---- fill-chain emitters (each is one dense PE burst) ----
            def v_chain(st):
                kb, c0 = st // 4, (st % 4) * P
                ps = fl_pool.tile([P, 512], F32, tag="fill", bufs=2, name="vps")
                for t in range(DT):
                    nc.tensor.matmul(
                        ps[:, :CL],
                        lhsT=xt_sb[:, kb, t, c0:c0 + P],
                        rhs=wv_sb[:, t, :],
                        start=(t == 0), stop=(t == DT - 1),
                    )
                # (GpSimd cannot read PSUM on TRN2 -> evacuate on DVE)
                nc.vector.tensor_copy(
                    v_sb[:, st, :, :HD],
                    ps[:, :CL].rearrange("p (h d) -> p h d", h=NH),
                )

            def qk_chain(which, ct, sg):  # which: 0=q, 1=k
                w_sb = (wq_sb, wk_sb)[which]
                dst = (qT_sb, kT_sb)[which]
                ps = fl_pool.tile([P, 512], F32, tag="fill", bufs=2, name="qkps")
                for t in range(DT):
                    nc.tensor.matmul(
                        ps,
                        lhsT=w_sb[:, t, ct * P:(ct + 1) * P],
                        rhs=xt_sb[:, sg, t, :],
                        start=(t == 0), stop=(t == DT - 1),
                    )
                nc.vector.tensor_tensor(
                    out=dst[:, ct, sg * 512:(sg + 1) * 512],
                    in0=ps,
                    in1=b_sb[:, which, ct:ct + 1].to_broadcast((P, 512)),
                    op=AluOpType.add,
                )

            def o_chain(st):
                osb = os_pool.tile([P, D], F16, tag="osb", bufs=3, name="osb")
                for ng in range(2):
                    ps = fl_pool.tile([P, 512], F32, tag="fill", bufs=2,
                                      name="ops")
                    for ct in range(CT):
                        nc.tensor.matmul(
                            ps,
                            lhsT=aT_ct[ct][:, st * P:(st + 1) * P],
                            rhs=wo_sb[:, ct, ng * 512:(ng + 1) * 512],
                            start=(ct == 0), stop=(ct == CT - 1),
                        )
                    nc.vector.tensor_copy(osb[:, ng * 512:(ng + 1) * 512], ps)
                nc.sync.dma_start(out=out[st * P:(st + 1) * P, :], in_=osb)

            # ---- attention group: heads in PAIRS (one ch-tile) ----
            # scoresT[j,i] via lhsT=kT, rhs=qT so post-exp weights are
            # already the attn@v moving operand -- no transposes.  Fixed
            # zero softmax shift (scores/8 ~ N(0,1), exp<=e^6, no overflow).
            def attn_group(pt, qg, fills, split_drain=False):
                njt = 4 * qg + 4
                at0 = at_pool.tile([P, 512], F32, tag="at", bufs=2, name="at0")
                at1 = at_pool.tile([P, 512], F32, tag="at", bufs=2, name="at1")
                wts = {}

                def emit_sc(jt):
                    r0 = max(0, jt - 4 * qg) * P   # first valid i col
                    sc = sc_pool.tile([P, 1024], F32, tag="sc", bufs=2,
                                      name="sc")
                    for hh, po in ((0, 0), (1, HD)):
                        nc.tensor.matmul(
                            sc[:, hh * 512 + r0:(hh + 1) * 512],
                            lhsT=kT_sb[po:po + HD, pt, jt * P:(jt + 1) * P],
                            rhs=qT_sb[po:po + HD, pt,
                                      qg * 512 + r0:(qg + 1) * 512],
                            start=True, stop=True,
                        )
                    wt = wt_pool.tile([P, 1024], CDT, tag="wt", bufs=4,
                                      name="wt")
                    nc.scalar.activation(
                        out=wt[:, r0:], in_=sc[:, r0:],
                        func=mybir.ActivationFunctionType.Exp,
                        scale=float(SCALE),
                    )
                    if jt >= 4 * qg:
                        # diagonal block: zero weights above the causal diag
                        # (iota = i_local - j_local; one op per head).
                        for hh in range(2):
                            c0 = hh * 512 + r0
                            nc.gpsimd.affine_select(
                                out=wt[:, c0:c0 + P], in_=wt[:, c0:c0 + P],
                                compare_op=AluOpType.is_ge,
                                fill=0.0, base=0, pattern=[[1, P]],
                                channel_multiplier=-1,
                            )
                    wts[jt] = (wt, r0)

                emit_sc(0)
                for jt in range(njt):
                    if jt + 1 < njt:
                        emit_sc(jt + 1)
                    for f in fills.get(jt, ()):
                        f()
                    wt, r0 = wts.pop(jt)
                    for hh, at in ((0, at0), (1, at1)):
                        nc.tensor.matmul(
                            at[:, r0:],
                            lhsT=v_sb[:, jt, 2 * pt + hh, :],
                            rhs=wt[:, hh * 512 + r0:(hh + 1) * 512],
                            start=(jt == 0), stop=(jt == njt - 1),
                        )
                # drain: quick PSUM evacuation (one copy per head frees the
                # bank), then the elastic normalize.  Both heads land in one
                # [128,2,512] tile so rows 64:128 give the two denominator
                # sets as a contiguous [64,1024] block; 1/d is computed as
                # exp(-ln(d)) on the Scalar engine -- ln and exp live in the
                # SAME activation table ('natural_log_exp_and_others') as the
                # softmax exps, so no 1.5us ACT table reloads, unlike
                # Reciprocal (own table) or DVE reciprocal (3.9us each).
                asb = sm_pool.tile([P, 2, 512], F32, tag="asb", bufs=3,
                                   name="asb")
                rd = sm_pool.tile([HD, 2, 512], F32, tag="rd", bufs=2,
                                  name="rd")
                rd2 = sm_pool.tile([HD, 2, 512], F32, tag="rd2", bufs=2,
                                   name="rd2")

                def _mult(hh):
                    nc.vector.tensor_tensor(
                        out=aT_ct[pt][hh * HD:(hh + 1) * HD,
                                      qg * 512:(qg + 1) * 512],
                        in0=asb[:HD, hh, :], in1=rd2[:, hh, :],
                        op=AluOpType.mult,
                    )

                if split_drain:
                    # last group: copies first (frees PSUM, keeps DVE queue
                    # unblocked), then per-head ln/exp/mult so the first
                    # head's normalized aT unblocks the epilogue early.
                    for hh, at in ((0, at0), (1, at1)):
                        nc.vector.tensor_copy(asb[:, hh, :], at)
                    for hh in (0, 1):
                        nc.scalar.activation(
                            out=rd[:, hh, :], in_=asb[HD:2 * HD, hh, :],
                            func=mybir.ActivationFunctionType.Ln,
                        )
                        nc.scalar.activation(
                            out=rd2[:, hh, :], in_=rd[:, hh, :],
                            func=mybir.ActivationFunctionType.Exp, scale=-1.0,
                        )
                        _mult(hh)
                else:
                    for hh, at in ((0, at0), (1, at1)):
                        nc.vector.tensor_copy(asb[:, hh, :], at)
                    nc.scalar.activation(
                        out=rd, in_=asb[HD:2 * HD, :, :],
                        func=mybir.ActivationFunctionType.Ln,
                    )
                    nc.scalar.activation(
                        out=rd2, in_=rd,
                        func=mybir.ActivationFunctionType.Exp, scale=-1.0,
                    )
                    _mult(0)
                    _mult(1)

            # ---- prologue: first v/qk chains (need only xt block 0) ----
            for st in range(4):
                v_chain(st)
            qk_chain(0, 0, 0)
            qk_chain(1, 0, 0)

            # ---- attention groups with interleaved fills ----
            # o(st) needs attn qg=st//4 drained for BOTH pts; qk(sg) feeds
            # sc of groups with qg>=sg; v(st) feeds at of j-tile st.  Later
            # groups get finer-grained fills (half o-chains) so every j-tile
            # carries ~200ns of independent PE work and the tensor engine
            # never resets its p-state ramp.
            osbs = {}

            def o_half(st, ng):
                if ng == 0:
                    osbs[st] = os_pool.tile([P, D], F16, tag="osb", bufs=3,
                                            name="osb")
                osb = osbs[st]
                ps = fl_pool.tile([P, 512], F32, tag="fill", bufs=2, name="ops")
                for ct in range(CT):
                    nc.tensor.matmul(
                        ps,
                        lhsT=aT_ct[ct][:, st * P:(st + 1) * P],
                        rhs=wo_sb[:, ct, ng * 512:(ng + 1) * 512],
                        start=(ct == 0), stop=(ct == CT - 1),
                    )
                nc.vector.tensor_copy(osb[:, ng * 512:(ng + 1) * 512], ps)
                if ng == 1:
                    nc.sync.dma_start(out=out[st * P:(st + 1) * P, :], in_=osb)

            attn_group(0, 0, {0: [lambda: qk_chain(0, 1, 0)],
                              1: [lambda: qk_chain(1, 1, 0)],
                              2: [lambda: v_chain(4)],
                              3: [lambda: v_chain(5)]})
            attn_group(1, 0, {0: [lambda: qk_chain(0, 0, 1)],
                              1: [lambda: qk_chain(1, 0, 1)],
                              2: [lambda: v_chain(6)],
                              3: [lambda: v_chain(7)]})
            attn_group(0, 1, {0: [lambda: qk_chain(0, 1, 1)],
                              1: [lambda: qk_chain(1, 1, 1)],
                              3: [lambda: o_chain(0)],
                              6: [lambda: o_chain(1)]})
            attn_group(1, 1, {0: [lambda: qk_chain(0, 0, 2)],
                              1: [lambda: qk_chain(1, 0, 2)],
                              3: [lambda: o_chain(2)],
                              6: [lambda: o_chain(3)]})
            attn_group(0, 2, {0: [lambda: qk_chain(0, 1, 2)],
                              1: [lambda: qk_chain(1, 1, 2)],
                              2: [lambda: v_chain(8)],
                              3: [lambda: v_chain(9)],
                              5: [lambda: v_chain(10)],
                              7: [lambda: v_chain(11)],
                              9: [lambda: o_chain(4)],
                              11: [lambda: o_chain(5)]})
            attn_group(1, 2, {0: [lambda: qk_chain(0, 0, 3)],
                              2: [lambda: qk_chain(1, 0, 3)],
                              5: [lambda: o_chain(6)],
                              9: [lambda: o_chain(7)]})
            attn_group(0, 3, {0: [lambda: qk_chain(0, 1, 3)],
                              2: [lambda: qk_chain(1, 1, 3)],
                              4: [lambda: v_chain(12)],
                              6: [lambda: v_chain(13)],
                              8: [lambda: v_chain(14)],
                              10: [lambda: v_chain(15)],
                              12: [lambda: o_half(8, 0)],
                              14: [lambda: o_half(8, 1)]})
            attn_group(1, 3, {1: [lambda: o_half(9, 0)],
                              3: [lambda: o_half(9, 1)],
                              5: [lambda: o_half(10, 0)],
                              7: [lambda: o_half(10, 1)],
                              9: [lambda: o_half(11, 0)],
                              11: [lambda: o_half(11, 1)]},
                       split_drain=True)

            # ---- epilogue: last output-projection blocks ----
            for st in range(12, 16):
                o_chain(st)

    _legalize_waits(nc)
    return nc


_NC_CACHE = {}


def _get_nc():
    if "nc" not in _NC_CACHE:
        _NC_CACHE["nc"] = build_nc()
    return _NC_CACHE["nc"]


def make_in_maps(x, Wq, bq, Wk, bk, Wv, bv, Wo, bo):
    np_cdt = ml_dtypes.bfloat16 if CDT == BF16 else np.float32
    x32 = np.asarray(x, np.float32)
    Wq32 = np.asarray(Wq, np.float32)
    Wk32 = np.asarray(Wk, np.float32)
    Wv32 = np.asarray(Wv, np.float32)
    Wo32 = np.asarray(Wo, np.float32)
    bq32 = np.asarray(bq, np.float32)
    bk32 = np.asarray(bk, np.float32)

    def pack_w(W):  # [D, CL] -> [P, DT*CL], row p = concat_t W[t*P+p, :]
        return np.ascontiguousarray(
            W.reshape(DT, P, CL).transpose(1, 0, 2).reshape(P, DT * CL)
        ).astype(np_cdt)

    in_maps = []
    for c in range(8):
        b, hg = divmod(c, 4)
        cs = slice(hg * CL, (hg + 1) * CL)
        xtT = np.ascontiguousarray(x32[b].T)  # [D, S]
        xtp = np.ascontiguousarray(
            xtT.reshape(DT, P, XB, 512).transpose(1, 2, 0, 3)
            .reshape(P, XB * DT * 512)
        ).astype(np_cdt)
        wop = np.ascontiguousarray(
            Wo32[cs, :].reshape(CT, P, D).transpose(1, 0, 2).reshape(P, CT * D)
        ).astype(np_cdt)
        bqkp = np.ascontiguousarray(
            np.stack([bq32[cs], bk32[cs]]).reshape(2, CT, P)
            .transpose(2, 0, 1).reshape(P, 2 * CT)
        )
        in_maps.append({
            "xt": xtp,
            "wq": pack_w(Wq32[:, cs]),
            "wk": pack_w(Wk32[:, cs]),
            "wv": pack_w(Wv32[:, cs]),
            "wo": wop,
            "bqk": bqkp,
        })
    return in_maps


def run_spmd(in_maps, **kw):
    from concourse.bass_utils import run_bass_kernel_spmd
    return run_bass_kernel_spmd(_get_nc(), in_maps, core_ids=list(range(8)), **kw)


def gather(results, bv, Wo, bo):
    bo = np.asarray(bo, np.float32)
    bv = np.asarray(bv, np.float32)
    Wo = np.asarray(Wo, np.float32)
    corr = bo + bv @ Wo  # exact: softmax rows sum to 1, so attn(v+bv)=attn(v)+bv
    out = np.empty((2, S, D), np.float32)
    for b in range(2):
        acc = results[4 * b]["out"].astype(np.float32)
        for i in range(1, 4):
            acc = acc + results[4 * b + i]["out"].astype(np.float32)
        out[b] = acc + corr
    return out


def kernel(x, Wq, bq, Wk, bk, Wv, bv, Wo, bo):
    in_maps = make_in_maps(x, Wq, bq, Wk, bk, Wv, bv, Wo, bo)
    res = run_spmd(in_maps)
    return gather(res.results, bv, Wo, bo)


# revision 25
# speedup vs baseline: 1.0001x; 1.0001x over previous
"""Multi-head self-attention (causal) on 8 TRN2 NeuronCores — v2.

Problem (hardcoded): B=2, S=2048, D=1024, H=16 heads, HD=64.
  q,k,v = x@W* + b*; scores = qk^T/sqrt(HD) causal-masked; softmax;
  out = (softmax @ v) @ Wo + bo.

Sharding: 8 cores = 2 batches x 4 head-groups (4 heads each).
Core c handles batch c//4, heads (c%4)*4..(c%4)*4+4 (Megatron-style TP:
Wq/Wk/Wv column-sliced, Wo row-sliced; host sums the 4 partial outputs
per batch and adds bo + bv@Wo -- the bv fold is exact because softmax
rows sum to 1, so attn(v+bv) = attn(v) + bv).

v2 is scheduled around the TRN2 PE p-state ramp: the tensor engine only
reaches 2.4 GHz after ~3us of *gapless* execution and drops to 1.2 GHz
after any idle (the v1 trace shows identical matmuls at 216ns in the
dense projection phase vs 427-585ns in the gappy attention phase).  So
the whole kernel is emitted as ONE dense PE stream:

  - scores for j-tile jt+1 are issued before attn@v of j-tile jt, so
    the PE never sits behind the Scalar-engine exp;
  - v/q/k/out-projection matmul chains are interleaved into the
    attention j-tile loop as elastic "fill" work that plugs the
    remaining exp-lag (ACT needs ~1.01us per j-tile vs ~0.85us of
    attention PE work at full clock);
  - x is host-packed so it loads in four 512-column blocks (8KB
    contiguous per partition row) and the first v/q/k chains start
    after ~1.5MB of DMA instead of ~6MB.

Other changes vs v1: causal masking is a post-exp affine_select on the
(otherwise idle) GpSimd engine instead of -1e9 adds on Vector (exp of
an unmasked score is <=e^6, far from overflow, and zeroing the weight
kills the contribution *and* keeps the ones-column denominator exact);
the softmax denominator reciprocal uses reciprocal_approx_fast (~5x
faster than v1's 3.3us InstReciprocal, 18-bit accurate); PSUM attn
accumulators are evacuated with one quick copy so the banks recycle
fast, and the normalize multiply runs on GpSimd; the output is written
as fp16 (halves the store traffic; partials are ~O(3) so fp16 adds
<0.1% error).
"""

import numpy as np
import ml_dtypes

import concourse.bass as bass
import concourse.mybir as mybir
import concourse.tile as tile
from concourse.alu_op_type import AluOpType

P = 128
S = 2048          # per-core sequence (one batch slice)
D = 1024
CL = 256          # local channels = 4 heads * 64
NH = 4            # local heads
HD = 64
DT = D // P       # 8 contraction chunks
CT = CL // P      # 2 local-channel tiles
ST = S // P       # 16 seq tiles
QG = 4            # 512-wide query groups
XB = 4            # xt DMA column blocks of 512
SCALE = 1.0 / np.sqrt(HD)

F32 = mybir.dt.float32
F16 = mybir.dt.float16
BF16 = mybir.dt.bfloat16
CDT = BF16        # compute dtype for matmul operands


def _legalize_waits(nc: bass.Bass) -> None:
    """Hoist excess sync waits into standalone EventSemaphore instructions.

    The TRN2 ISA holds ONE sync-wait per instruction (two on
    EventSemaphore); Tile's sem-assignment can attach more, which walrus
    rejects with "Too many sync wait commands".  Executing the extra
    waits as same-engine EventSemaphores immediately before the
    instruction is semantically identical.
    """
    esn = 0
    for fn in nc.m.functions:
        for blk in fn.blocks:
            new = []
            for inst in blk.instructions:
                si = inst.sync_info
                cap = 2 if isinstance(inst, mybir.InstEventSemaphore) else 1
                if si is not None and si.on_wait and len(si.on_wait) > cap:
                    waits = list(si.on_wait)
                    extra, keep = waits[:-cap], waits[-cap:]
                    while extra:
                        chunk, extra = extra[:2], extra[2:]
                        esn += 1
                        new.append(mybir.InstEventSemaphore(
                            name=f"eswait{esn}_{inst.name}",
                            engine=inst.engine, ins=[], outs=[],
                            sync_info=mybir.SyncInfo(on_wait=chunk, on_update=[]),
                        ))
                    inst.sync_info = mybir.SyncInfo(
                        on_wait=keep, on_update=list(si.on_update)
                    )
                new.append(inst)
            blk.instructions[:] = new


def build_nc() -> bass.Bass:
    nc = bass.Bass()
    xt = nc.declare_dram_parameter("xt", [P, XB * DT * 512], CDT, isOutput=False)
    wq = nc.declare_dram_parameter("wq", [P, DT * CL], CDT, isOutput=False)
    wk = nc.declare_dram_parameter("wk", [P, DT * CL], CDT, isOutput=False)
    wv = nc.declare_dram_parameter("wv", [P, DT * CL], CDT, isOutput=False)
    wo = nc.declare_dram_parameter("wo", [P, CT * D], CDT, isOutput=False)
    bqk = nc.declare_dram_parameter("bqk", [P, 2 * CT], F32, isOutput=False)
    out = nc.declare_dram_parameter("out", [S, D], F16, isOutput=True)

    with tile.TileContext(nc) as tc:
        with tc.tile_pool(name="const", bufs=1) as const, \
             tc.tile_pool(name="sc_ps", bufs=2, space="PSUM") as sc_pool, \
             tc.tile_pool(name="at_ps", bufs=2, space="PSUM") as at_pool, \
             tc.tile_pool(name="fl_ps", bufs=2, space="PSUM") as fl_pool, \
             tc.tile_pool(name="wtp", bufs=4) as wt_pool, \
             tc.tile_pool(name="smp", bufs=4) as sm_pool, \
             tc.tile_pool(name="osp", bufs=3) as os_pool:

            # persistent SBUF tensors
            # xt is BLOCK-major ([P, 4 col-blocks, 8 k-chunks, 512]) so each
            # block's DMA writes one contiguous 8KB run per partition: the
            # SP sequencer generates 128 descriptors (~0.6us) instead of
            # 1024 (~4.4us) per block, and compute starts ~7us earlier.
            xt_sb = const.tile([P, XB, DT, 512], CDT)
            wq_sb = const.tile([P, DT, CL], CDT)
            wk_sb = const.tile([P, DT, CL], CDT)
            wv_sb = const.tile([P, DT, CL], CDT)
            wo_sb = const.tile([P, CT, D], CDT)
            b_sb = const.tile([P, 2, CT], F32)
            qT_sb = const.tile([P, CT, S], CDT)
            kT_sb = const.tile([P, CT, S], CDT)
            # cols [HD, 2*HD) are all-ones: the attn matmul then emits the
            # softmax denominator replicated on PSUM partitions 64..127.
            v_sb = const.tile([P, ST, NH, 2 * HD], CDT)
            # attnT (normalized), SPLIT per ct: a shared tile made the
            # epilogue's ct0 matmuls inherit a false wait on the final
            # group's ct1 normalize (conservative cross-ct dependency).
            aT0_sb = const.tile([P, S], CDT)
            aT1_sb = const.tile([P, S], CDT)
            aT_ct = (aT0_sb, aT1_sb)
            escr = const.tile([P, 2], F32)                # exp-table preload

            # ---- input DMAs, priority order ----
            # gpsimd = single SWDGE queue for the tiny bias gather.
            b_ld = const.tile([P, 2, CT], F32)
            nc.gpsimd.dma_start(
                out=b_ld[:], in_=bqk.rearrange("p (w c) -> p w c", w=2)
            )
            # first-needed loads first: the prologue v/qk chains need only
            # wv+wq+wk and xt block 0.
            xt_dr = xt.rearrange("p (k t j) -> p k t j", k=XB, t=DT)
            nc.sync.dma_start(
                out=wv_sb[:], in_=wv.rearrange("p (t c) -> p t c", t=DT)
            )
            nc.sync.dma_start(out=xt_sb[:, 0], in_=xt_dr[:, 0])
            for w_sb, w_dr in ((wq_sb, wq), (wk_sb, wk)):
                nc.sync.dma_start(
                    out=w_sb[:], in_=w_dr.rearrange("p (t c) -> p t c", t=DT)
                )
            for k in range(1, XB):
                nc.sync.dma_start(out=xt_sb[:, k], in_=xt_dr[:, k])
            nc.sync.dma_start(
                out=wo_sb[:], in_=wo.rearrange("p (t c) -> p t c", t=CT)
            )

            # TensorScalarPtr holds only ONE sync wait, so absorb the DMA
            # wait into a DVE copy: consumers then only wait on DVE.
            nc.vector.tensor_copy(b_sb[:], b_ld[:])
            nc.vector.memset(v_sb[:, :, :, HD:], 1.0)
            # preload the Exp table while DMAs stream (one-time 1.3us).
            nc.scalar.activation(
                out=escr, in_=b_sb[:, 0, :],
                func=mybir.ActivationFunctionType.Exp,
            )

            # ---- fill-chain emitters (each is one dense PE burst) ----
            def v_chain(st):
                kb, c0 = st // 4, (st % 4) * P
                ps = fl_pool.tile([P, 512], F32, tag="fill", bufs=2, name="vps")
                for t in range(DT):
                    nc.tensor.matmul(
                        ps[:, :CL],
                        lhsT=xt_sb[:, kb, t, c0:c0 + P],
                        rhs=wv_sb[:, t, :],
                        start=(t == 0), stop=(t == DT - 1),
                    )
                # (GpSimd cannot read PSUM on TRN2 -> evacuate on DVE)
                nc.vector.tensor_copy(
                    v_sb[:, st, :, :HD],
                    ps[:, :CL].rearrange("p (h d) -> p h d", h=NH),
                )

            def qk_chain(which, ct, sg):  # which: 0=q, 1=k
                w_sb = (wq_sb, wk_sb)[which]
                dst = (qT_sb, kT_sb)[which]
                ps = fl_pool.tile([P, 512], F32, tag="fill", bufs=2, name="qkps")
                for t in range(DT):
                    nc.tensor.matmul(
                        ps,
                        lhsT=w_sb[:, t, ct * P:(ct + 1) * P],
                        rhs=xt_sb[:, sg, t, :],
                        start=(t == 0), stop=(t == DT - 1),
                    )
                nc.vector.tensor_tensor(
                    out=dst[:, ct, sg * 512:(sg + 1) * 512],
                    in0=ps,
                    in1=b_sb[:, which, ct:ct + 1].to_broadcast((P, 512)),
                    op=AluOpType.add,
                )

            def o_chain(st):
                osb = os_pool.tile([P, D], F16, tag="osb", bufs=3, name="osb")
                for ng in range(2):
                    ps = fl_pool.tile([P, 512], F32, tag="fill", bufs=2,
                                      name="ops")
                    for ct in range(CT):
                        nc.tensor.matmul(
                            ps,
                            lhsT=aT_ct[ct][:, st * P:(st + 1) * P],
                            rhs=wo_sb[:, ct, ng * 512:(ng + 1) * 512],
                            start=(ct == 0), stop=(ct == CT - 1),
                        )
                    nc.vector.tensor_copy(osb[:, ng * 512:(ng + 1) * 512], ps)
                nc.sync.dma_start(out=out[st * P:(st + 1) * P, :], in_=osb)

            # ---- attention group: heads in PAIRS (one ch-tile) ----
            # scoresT[j,i] via lhsT=kT, rhs=qT so post-exp weights are
            # already the attn@v moving operand -- no transposes.  Fixed
            # zero softmax shift (scores/8 ~ N(0,1), exp<=e^6, no overflow).
            def attn_group(pt, qg, fills, split_drain=False):
                njt = 4 * qg + 4
                at0 = at_pool.tile([P, 512], F32, tag="at", bufs=2, name="at0")
                at1 = at_pool.tile([P, 512], F32, tag="at", bufs=2, name="at1")
                wts = {}

                def emit_sc(jt):
                    r0 = max(0, jt - 4 * qg) * P   # first valid i col
                    sc = sc_pool.tile([P, 1024], F32, tag="sc", bufs=2,
                                      name="sc")
                    for hh, po in ((0, 0), (1, HD)):
                        nc.tensor.matmul(
                            sc[:, hh * 512 + r0:(hh + 1) * 512],
                            lhsT=kT_sb[po:po + HD, pt, jt * P:(jt + 1) * P],
                            rhs=qT_sb[po:po + HD, pt,
                                      qg * 512 + r0:(qg + 1) * 512],
                            start=True, stop=True,
                        )
                    wt = wt_pool.tile([P, 1024], CDT, tag="wt", bufs=4,
                                      name="wt")
                    nc.scalar.activation(
                        out=wt[:, r0:], in_=sc[:, r0:],
                        func=mybir.ActivationFunctionType.Exp,
                        scale=float(SCALE),
                    )
                    if jt >= 4 * qg:
                        # diagonal block: zero weights above the causal diag
                        # (iota = i_local - j_local; one op per head).
                        for hh in range(2):
                            c0 = hh * 512 + r0
                            nc.gpsimd.affine_select(
                                out=wt[:, c0:c0 + P], in_=wt[:, c0:c0 + P],
                                compare_op=AluOpType.is_ge,
                                fill=0.0, base=0, pattern=[[1, P]],
                                channel_multiplier=-1,
                            )
                    wts[jt] = (wt, r0)

                emit_sc(0)
                for jt in range(njt):
                    if jt + 1 < njt:
                        emit_sc(jt + 1)
                    for f in fills.get(jt, ()):
                        f()
                    wt, r0 = wts.pop(jt)
                    for hh, at in ((0, at0), (1, at1)):
                        nc.tensor.matmul(
                            at[:, r0:],
                            lhsT=v_sb[:, jt, 2 * pt + hh, :],
                            rhs=wt[:, hh * 512 + r0:(hh + 1) * 512],
                            start=(jt == 0), stop=(jt == njt - 1),
                        )
                # drain: quick PSUM evacuation (one copy per head frees the
                # bank), then the elastic normalize.  Both heads land in one
                # [128,2,512] tile so rows 64:128 give the two denominator
                # sets as a contiguous [64,1024] block; 1/d is computed as
                # exp(-ln(d)) on the Scalar engine -- ln and exp live in the
                # SAME activation table ('natural_log_exp_and_others') as the
                # softmax exps, so no 1.5us ACT table reloads, unlike
                # Reciprocal (own table) or DVE reciprocal (3.9us each).
                asb = sm_pool.tile([P, 2, 512], F32, tag="asb", bufs=3,
                                   name="asb")
                rd = sm_pool.tile([HD, 2, 512], F32, tag="rd", bufs=2,
                                  name="rd")
                rd2 = sm_pool.tile([HD, 2, 512], F32, tag="rd2", bufs=2,
                                   name="rd2")

                def _mult(hh):
                    nc.vector.tensor_tensor(
                        out=aT_ct[pt][hh * HD:(hh + 1) * HD,
                                      qg * 512:(qg + 1) * 512],
                        in0=asb[:HD, hh, :], in1=rd2[:, hh, :],
                        op=AluOpType.mult,
                    )

                if split_drain:
                    # last group: normalize straight from PSUM (no evacuation
                    # copy -- nothing else needs the banks after this), per
                    # head, so the epilogue unblocks ~1.5us earlier.  ln can
                    # start on head 0 while head 1's last attn matmul runs.
                    for hh, at in ((0, at0), (1, at1)):
                        nc.scalar.activation(
                            out=rd[:, hh, :], in_=at[HD:2 * HD, :],
                            func=mybir.ActivationFunctionType.Ln,
                        )
                        nc.scalar.activation(
                            out=rd2[:, hh, :], in_=rd[:, hh, :],
                            func=mybir.ActivationFunctionType.Exp, scale=-1.0,
                        )
                        nc.vector.tensor_tensor(
                            out=aT_ct[pt][hh * HD:(hh + 1) * HD,
                                          qg * 512:(qg + 1) * 512],
                            in0=at[:HD, :], in1=rd2[:, hh, :],
                            op=AluOpType.mult,
                        )
                else:
                    for hh, at in ((0, at0), (1, at1)):
                        nc.vector.tensor_copy(asb[:, hh, :], at)
                    nc.scalar.activation(
                        out=rd, in_=asb[HD:2 * HD, :, :],
                        func=mybir.ActivationFunctionType.Ln,
                    )
                    nc.scalar.activation(
                        out=rd2, in_=rd,
                        func=mybir.ActivationFunctionType.Exp, scale=-1.0,
                    )
                    _mult(0)
                    _mult(1)

            # ---- prologue: first v/qk chains (need only xt block 0) ----
            for st in range(4):
                v_chain(st)
            qk_chain(0, 0, 0)
            qk_chain(1, 0, 0)

            # ---- attention groups with interleaved fills ----
            # o(st) needs attn qg=st//4 drained for BOTH pts; qk(sg) feeds
            # sc of groups with qg>=sg; v(st) feeds at of j-tile st.  Later
            # groups get finer-grained fills (half o-chains) so every j-tile
            # carries ~200ns of independent PE work and the tensor engine
            # never resets its p-state ramp.
            osbs = {}

            def o_half(st, ng):
                if ng == 0:
                    osbs[st] = os_pool.tile([P, D], F16, tag="osb", bufs=3,
                                            name="osb")
                osb = osbs[st]
                ps = fl_pool.tile([P, 512], F32, tag="fill", bufs=2, name="ops")
                for ct in range(CT):
                    nc.tensor.matmul(
                        ps,
                        lhsT=aT_ct[ct][:, st * P:(st + 1) * P],
                        rhs=wo_sb[:, ct, ng * 512:(ng + 1) * 512],
                        start=(ct == 0), stop=(ct == CT - 1),
                    )
                nc.vector.tensor_copy(osb[:, ng * 512:(ng + 1) * 512], ps)
                if ng == 1:
                    nc.sync.dma_start(out=out[st * P:(st + 1) * P, :], in_=osb)

            def keepwarm(n):
                # real-but-unread matmuls that bridge dependency stalls so
                # the tensor engine's p-state ramp never resets (a ~2us idle
                # halves the clock for the next ~3us of matmuls).
                ps = fl_pool.tile([P, 512], F32, tag="fill", bufs=2,
                                  name="warm")
                for i in range(n):
                    nc.tensor.matmul(
                        ps, lhsT=xt_sb[:, 0, i % DT, :P],
                        rhs=wo_sb[:, 0, :512], start=True, stop=True,
                    )

            attn_group(0, 0, {0: [lambda: qk_chain(0, 1, 0)],
                              1: [lambda: qk_chain(1, 1, 0)],
                              2: [lambda: v_chain(4)],
                              3: [lambda: v_chain(5)]})
            attn_group(1, 0, {0: [lambda: qk_chain(0, 0, 1)],
                              1: [lambda: qk_chain(1, 0, 1)],
                              2: [lambda: v_chain(6)],
                              3: [lambda: v_chain(7)]})
            attn_group(0, 1, {0: [lambda: qk_chain(0, 1, 1)],
                              1: [lambda: qk_chain(1, 1, 1)],
                              3: [lambda: o_chain(0)],
                              6: [lambda: o_chain(1)]})
            attn_group(1, 1, {0: [lambda: qk_chain(0, 0, 2)],
                              1: [lambda: qk_chain(1, 0, 2)],
                              3: [lambda: o_chain(2)],
                              6: [lambda: o_chain(3)]})
            attn_group(0, 2, {0: [lambda: qk_chain(0, 1, 2)],
                              1: [lambda: qk_chain(1, 1, 2)],
                              2: [lambda: v_chain(8)],
                              3: [lambda: v_chain(9)],
                              5: [lambda: v_chain(10)],
                              7: [lambda: v_chain(11)],
                              9: [lambda: o_chain(4)],
                              11: [lambda: o_chain(5)]})
            attn_group(1, 2, {0: [lambda: qk_chain(0, 0, 3)],
                              2: [lambda: qk_chain(1, 0, 3)],
                              5: [lambda: o_chain(6)],
                              9: [lambda: o_chain(7)]})
            attn_group(0, 3, {0: [lambda: qk_chain(0, 1, 3)],
                              2: [lambda: qk_chain(1, 1, 3)],
                              4: [lambda: v_chain(12)],
                              6: [lambda: v_chain(13)],
                              8: [lambda: v_chain(14)],
                              10: [lambda: v_chain(15)],
                              12: [lambda: o_half(8, 0)],
                              14: [lambda: o_half(8, 1)]})
            attn_group(1, 3, {1: [lambda: o_half(9, 0)],
                              3: [lambda: o_half(9, 1)],
                              5: [lambda: o_half(10, 0)],
                              7: [lambda: o_half(10, 1)],
                              9: [lambda: o_half(11, 0)],
                              11: [lambda: o_half(11, 1)],
                              13: [lambda: keepwarm(2)]},
                       split_drain=True)

            # ---- epilogue: last output-projection blocks ----
            keepwarm(12)    # cover the final normalize (~2.5us)
            for st in range(12, 16):
                o_chain(st)

    _legalize_waits(nc)
    return nc


_NC_CACHE = {}


def _get_nc():
    if "nc" not in _NC_CACHE:
        _NC_CACHE["nc"] = build_nc()
    return _NC_CACHE["nc"]


def make_in_maps(x, Wq, bq, Wk, bk, Wv, bv, Wo, bo):
    np_cdt = ml_dtypes.bfloat16 if CDT == BF16 else np.float32
    x32 = np.asarray(x, np.float32)
    Wq32 = np.asarray(Wq, np.float32)
    Wk32 = np.asarray(Wk, np.float32)
    Wv32 = np.asarray(Wv, np.float32)
    Wo32 = np.asarray(Wo, np.float32)
    bq32 = np.asarray(bq, np.float32)
    bk32 = np.asarray(bk, np.float32)

    def pack_w(W):  # [D, CL] -> [P, DT*CL], row p = concat_t W[t*P+p, :]
        return np.ascontiguousarray(
            W.reshape(DT, P, CL).transpose(1, 0, 2).reshape(P, DT * CL)
        ).astype(np_cdt)

    in_maps = []
    for c in range(8):
        b, hg = divmod(c, 4)
        cs = slice(hg * CL, (hg + 1) * CL)
        xtT = np.ascontiguousarray(x32[b].T)  # [D, S]
        xtp = np.ascontiguousarray(
            xtT.reshape(DT, P, XB, 512).transpose(1, 2, 0, 3)
            .reshape(P, XB * DT * 512)
        ).astype(np_cdt)
        wop = np.ascontiguousarray(
            Wo32[cs, :].reshape(CT, P, D).transpose(1, 0, 2).reshape(P, CT * D)
        ).astype(np_cdt)
        bqkp = np.ascontiguousarray(
            np.stack([bq32[cs], bk32[cs]]).reshape(2, CT, P)
            .transpose(2, 0, 1).reshape(P, 2 * CT)
        )
        in_maps.append({
            "xt": xtp,
            "wq": pack_w(Wq32[:, cs]),
            "wk": pack_w(Wk32[:, cs]),
            "wv": pack_w(Wv32[:, cs]),
            "wo": wop,
            "bqk": bqkp,
        })
    return in_maps


def run_spmd(in_maps, **kw):
    from concourse.bass_utils import run_bass_kernel_spmd
    return run_bass_kernel_spmd(_get_nc(), in_maps, core_ids=list(range(8)), **kw)


def gather(results, bv, Wo, bo):
    bo = np.asarray(bo, np.float32)
    bv = np.asarray(bv, np.float32)
    Wo = np.asarray(Wo, np.float32)
    corr = bo + bv @ Wo  # exact: softmax rows sum to 1, so attn(v+bv)=attn(v)+bv
    out = np.empty((2, S, D), np.float32)
    for b in range(2):
        acc = results[4 * b]["out"].astype(np.float32)
        for i in range(1, 4):
            acc = acc + results[4 * b + i]["out"].astype(np.float32)
        out[b] = acc + corr
    return out


def kernel(x, Wq, bq, Wk, bk, Wv, bv, Wo, bo):
    in_maps = make_in_maps(x, Wq, bq, Wk, bk, Wv, bv, Wo, bo)
    res = run_spmd(in_maps)
    return gather(res.results, bv, Wo, bo)
